# revision 44
# baseline (speedup 1.0000x reference)
"""Trainium2 Bass kernel for the BPR ragged-attention-pooling model.

Data-parallel across 8 NeuronCores: users (and their item-j segments) are
sharded 512/core; each core receives the piece rows feeding its unique-text
segments, its item-j pieces, and (redundantly) the item-i unique-text pieces
it needs, plus replicated small parameters. No collectives — the host slices
inputs and stitches outputs.

Segment semantics replicate the jax reference exactly: segment ids outside
[0, nseg) are dropped (the reference's int32 arange*nseg overflows for
piece_seg, so ~11k trailing pieces contribute nothing), and empty segments
pool to a zero vector (emulated with one dummy zero-piece so z=1).

Per-core pipeline, per 128-piece chunk of each 128-segment tile:
  logits   s = rowsum(X * v_bcast)              (DVE fused affine_mul_reduce)
  weights  e = exp(s [* 1/z_prev])              (ACT, per-partition scale)
  one-hot  A_e[p,j] = (iota[j]==segrel[p]) * e  (GPSIMD fused tensor_scalar)
  pool     psum += A_e.T @ X ; z += A_e.T @ 1   (PE matmuls)
Latents via PE transposes + W^T matmuls, predictions via DVE fused
mult-reduce. Pooling matmuls run as float32r (full PE rate; plain fp32
streams at 1/4 rate), costing ~3e-4 relative error. Softmax max-subtraction
is skipped: logits are O(0.2) here, so exp is safe and the result is
shift-invariant anyway.
"""

import sys

if "/opt/trn_rl_repo" not in sys.path:
    sys.path.insert(0, "/opt/trn_rl_repo")

import numpy as np

import concourse.bass as bass
import concourse.tile as tile
from concourse import bacc, mybir
from concourse.bass_utils import run_bass_kernel_spmd

F32 = mybir.dt.float32
F32R = mybir.dt.float32r
AF = mybir.ActivationFunctionType
ALU = mybir.AluOpType

D = 512          # model dim
FACTOR = 128
BS = 4096        # users
N_UNI = 24576    # unique user texts
P_U = 98304      # user piece rows
P_J = 16384      # item-j piece rows
NCORES = 8

U_C = BS // NCORES            # 512 users per core
T_U = U_C // 128              # 4 user seg-tiles per core
UNI_C = N_UNI // NCORES       # 3072 uni rows per core
T1 = UNI_C // 128             # 24 pass-1 seg-tiles per core
UNI_PER_TILE = UNI_C // T_U   # 768 uni rows per user tile
CH2 = UNI_PER_TILE // 128     # 6 pass-2 chunks per user tile

_CACHE = {}


def _build_program(ch1, chi, chj):
    nc = bacc.Bacc("TRN2", target_bir_lowering=False, debug=False)

    # ---- DRAM I/O ----
    X1 = nc.dram_tensor("X1", [T1 * ch1 * 128, D], F32R, kind="ExternalInput")
    XI = nc.dram_tensor("XI", [T_U * chi * 128, D], F32R, kind="ExternalInput")
    XJ = nc.dram_tensor("XJ", [T_U * chj * 128, D], F32R, kind="ExternalInput")
    SR1 = nc.dram_tensor("SR1", [128, T1 * ch1], F32, kind="ExternalInput")
    SRI = nc.dram_tensor("SRI", [128, T_U * chi], F32, kind="ExternalInput")
    SRJ = nc.dram_tensor("SRJ", [128, T_U * chj], F32, kind="ExternalInput")
    SR2 = nc.dram_tensor("SR2", [128, T1], F32, kind="ExternalInput")
    V0B = nc.dram_tensor("V0B", [128, D], F32, kind="ExternalInput")
    V1B = nc.dram_tensor("V1B", [128, D], F32, kind="ExternalInput")
    IOT = nc.dram_tensor("IOT", [128, 128], F32, kind="ExternalInput")
    WU = nc.dram_tensor("WU", [128, 512], F32, kind="ExternalInput")
    WIT = nc.dram_tensor("WIT", [128, 4, 128], F32, kind="ExternalInput")
    BUB = nc.dram_tensor("BUB", [128, 128], F32, kind="ExternalInput")
    BIB = nc.dram_tensor("BIB", [128, 128], F32, kind="ExternalInput")
    IDT = nc.dram_tensor("IDT", [128, 128], F32, kind="ExternalInput")
    ONE = nc.dram_tensor("ONE", [128, 2], F32R, kind="ExternalInput")

    itemi_o = nc.dram_tensor("itemi_o", [U_C, D], F32, kind="ExternalOutput")
    itemj_o = nc.dram_tensor("itemj_o", [U_C, D], F32, kind="ExternalOutput")
    # partition-major [p, tile] layout; the host transposes when stitching
    pred_i_o = nc.dram_tensor("pred_i_o", [128, T_U], F32, kind="ExternalOutput")
    pred_j_o = nc.dram_tensor("pred_j_o", [128, T_U], F32, kind="ExternalOutput")

    with tile.TileContext(nc) as tc:
        with (
            tc.tile_pool(name="persist", bufs=1) as pp,
            tc.tile_pool(name="xin", bufs=8) as xin,
            tc.tile_pool(name="prod", bufs=4) as prodp,
            tc.tile_pool(name="cols", bufs=6) as colp,
            tc.tile_pool(name="ae", bufs=3) as aep,
            tc.tile_pool(name="latT", bufs=4) as latTp,
            tc.tile_pool(name="lat", bufs=4) as latp,
            tc.tile_pool(name="mps", bufs=3, space=bass.MemorySpace.PSUM) as mps,
            tc.tile_pool(name="p2ps", bufs=1, space=bass.MemorySpace.PSUM) as p2ps,
            tc.tile_pool(name="z2ps", bufs=1, space=bass.MemorySpace.PSUM) as z2ps,
            tc.tile_pool(name="zps", bufs=1, space=bass.MemorySpace.PSUM) as zps,
            tc.tile_pool(name="tps", bufs=1, space=bass.MemorySpace.PSUM) as tps,
            tc.tile_pool(name="lps", bufs=1, space=bass.MemorySpace.PSUM) as lps,
        ):
            # ---- persistent SBUF ----
            v0b = pp.tile([128, D], F32)
            v1b = pp.tile([128, D], F32)
            sr1 = pp.tile([128, T1 * ch1], F32)
            sri = pp.tile([128, T_U * chi], F32)
            srj = pp.tile([128, T_U * chj], F32)
            sr2 = pp.tile([128, T1], F32)
            iot = pp.tile([128, 128], F32)
            wu = pp.tile([128, 512], F32)
            wit = pp.tile([128, 4, 128], F32)
            bub = pp.tile([128, 128], F32)
            bib = pp.tile([128, 128], F32)
            idt = pp.tile([128, 128], F32)
            ones = pp.tile([128, 2], F32R)
            uni_raw = pp.tile([128, T1, D], F32R)      # raw pass-1 sums
            z1c = pp.tile([128, T1, 2], F32R)          # pass-1 softmax denoms (paired)
            r1c = pp.tile([128, T1], F32)             # 1/z1
            itemiN = pp.tile([128, T_U, D], F32)      # normalized item-i rep
            itemjN = pp.tile([128, T_U, D], F32)
            r2c = pp.tile([128, T_U], F32)            # 1/z2
            pic = pp.tile([128, T_U], F32)            # pred_i columns
            pjc = pp.tile([128, T_U], F32)            # pred_j columns

            for dst, src in (
                (v0b, V0B), (v1b, V1B), (sr1, SR1), (sri, SRI), (srj, SRJ),
                (sr2, SR2), (iot, IOT), (wu, WU), (wit, WIT), (bub, BUB),
                (bib, BIB), (idt, IDT), (ones, ONE),
            ):
                nc.sync.dma_start(dst[:], src[:])

            def pool_tile(n_chunks, get_x, srcol, vb, get_zrhs, scale_col, store):
                """One 128-segment tile of softmax attention pooling."""
                ps = mps.tile([128, D], F32)
                zp = zps.tile([128, 2], F32)
                for k in range(n_chunks):
                    x = get_x(k)                      # [128, D] SBUF
                    prodt = prodp.tile([128, D], F32)
                    s = colp.tile([128, 1], F32)
                    nc.vector.affine_mul_reduce(
                        out=prodt[:], accum_out=s[:], in0=x, in1=vb[:],
                        scale=1.0, bias=0.0,
                    )
                    e = colp.tile([128, 1], F32)
                    if scale_col is None:
                        nc.scalar.activation(e[:], s[:], AF.Exp)
                    else:
                        # e2 = exp(s_raw / z1); then scale by 1/z1 again so
                        # the pooling matmul consumes normalized rows.
                        nc.scalar.activation(e[:], s[:], AF.Exp,
                                             scale=scale_col(k))
                        e2 = colp.tile([128, 1], F32)
                        nc.scalar.mul(e2[:], e[:], scale_col(k))
                        e = e2
                    # A_e[p, j] = (j == segrel[p]) * e[p]
                    ae = aep.tile([128, 128], F32R)
                    nc.gpsimd.tensor_scalar(ae[:], iot[:], srcol(k), e[:],
                                            ALU.is_equal, ALU.mult)
                    nc.tensor.matmul(ps[:], lhsT=ae[:], rhs=x,
                                     start=(k == 0), stop=(k == n_chunks - 1))
                    nc.tensor.matmul(zp[:], lhsT=ae[:], rhs=get_zrhs(k),
                                     start=(k == 0), stop=(k == n_chunks - 1))
                store(ps, zp)

            def lat_from(src3, t, wt, recip_col):
                """lat[u,f] = (src[u,:] [/z]) @ W^T for one user tile."""
                lp = lps.tile([128, FACTOR], F32)
                for k in range(4):
                    tp = tps.tile([128, 128], F32)
                    nc.tensor.transpose(tp[:], src3[:, t, 128 * k:128 * (k + 1)],
                                        idt[:])
                    lhsT = latTp.tile([128, 128], F32)
                    nc.scalar.copy(lhsT[:], tp[:])
                    nc.tensor.matmul(lp[:], lhsT=lhsT[:], rhs=wt[:, k, :],
                                     start=(k == 0), stop=(k == 3))
                out = latp.tile([128, FACTOR], F32)
                if recip_col is None:
                    nc.scalar.copy(out[:], lp[:])
                else:
                    nc.vector.tensor_scalar_mul(out[:], lp[:], recip_col)
                return out

            def stream_x(dram, ch):
                def get(tile_idx, k):
                    x = xin.tile([128, D], F32R)
                    g = ch * tile_idx + k
                    nc.sync.dma_start(x[:], dram[128 * g:128 * (g + 1), :])
                    return x[:]
                return get

            getx1 = stream_x(X1, ch1)
            getxi = stream_x(XI, chi)
            getxj = stream_x(XJ, chj)

            def item_i_tile(tg):
                def storei(ps, zp, tg=tg):
                    rc = colp.tile([128, 1], F32)
                    nc.vector.reciprocal(rc[:], zp[:, 0:1])
                    nc.scalar.mul(itemiN[:, tg, :], ps[:], rc[:])

                pool_tile(chi, lambda k, tg=tg: getxi(tg, k),
                          lambda k, tg=tg: sri[:, chi * tg + k:chi * tg + k + 1],
                          v0b, lambda k: ones[:], None, storei)

            def item_j_tile(tg):
                def storej(ps, zp, tg=tg):
                    rc = colp.tile([128, 1], F32)
                    nc.vector.reciprocal(rc[:], zp[:, 0:1])
                    nc.scalar.mul(itemjN[:, tg, :], ps[:], rc[:])

                pool_tile(chj, lambda k, tg=tg: getxj(tg, k),
                          lambda k, tg=tg: srj[:, chj * tg + k:chj * tg + k + 1],
                          v0b, lambda k: ones[:], None, storej)

            # item tiles are independent of pass 1/2; emit them one group
            # early so the final group's item latents overlap its pass-1 DMA
            item_i_tile(0)
            item_j_tile(0)

            for tg in range(T_U):
                # pass-2 accumulators for this user tile (fed incrementally
                # as each pass-1 tile completes, so the pass-2 DVE work
                # overlaps the DMA-bound pass-1 stream)
                ps2 = p2ps.tile([128, D], F32)
                zp2 = z2ps.tile([128, 2], F32)
                # ---- pass 1: user pieces -> unique-text vectors ----
                for kk in range(CH2):
                    t1 = CH2 * tg + kk

                    def store1(ps, zp, t1=t1):
                        nc.vector.reciprocal(r1c[:, t1:t1 + 1], zp[:, 0:1])
                        nc.scalar.copy(z1c[:, t1, :], zp[:])
                        nc.scalar.copy(uni_raw[:, t1, :], ps[:])

                    pool_tile(ch1, lambda k, t1=t1: getx1(t1, k),
                              lambda k, t1=t1: sr1[:, ch1 * t1 + k:ch1 * t1 + k + 1],
                              v0b, lambda k: ones[:], None, store1)

                    # ---- pass 2 chunk kk: unique texts -> user rep (raw) ----
                    x2 = uni_raw[:, t1, :]
                    prod2t = prodp.tile([128, D], F32)
                    s2 = colp.tile([128, 1], F32)
                    nc.vector.affine_mul_reduce(
                        out=prod2t[:], accum_out=s2[:], in0=x2, in1=v1b[:],
                        scale=1.0, bias=0.0,
                    )
                    e2a = colp.tile([128, 1], F32)
                    nc.scalar.activation(e2a[:], s2[:], AF.Exp,
                                         scale=r1c[:, t1:t1 + 1])
                    e2b = colp.tile([128, 1], F32)
                    nc.scalar.mul(e2b[:], e2a[:], r1c[:, t1:t1 + 1])
                    ae2 = aep.tile([128, 128], F32R)
                    nc.gpsimd.tensor_scalar(ae2[:], iot[:], sr2[:, t1:t1 + 1],
                                            e2b[:], ALU.is_equal, ALU.mult)
                    nc.tensor.matmul(ps2[:], lhsT=ae2[:], rhs=x2,
                                     start=(kk == 0), stop=(kk == CH2 - 1))
                    nc.tensor.matmul(zp2[:], lhsT=ae2[:], rhs=z1c[:, t1, :],
                                     start=(kk == 0), stop=(kk == CH2 - 1))

                if tg + 1 < T_U:
                    item_i_tile(tg + 1)
                    item_j_tile(tg + 1)

                # ---- pass 2 epilogue (ps2 stays live for the pred reduce) ----
                nc.vector.reciprocal(r2c[:, tg:tg + 1], zp2[:, 0:1])

                # ---- latents + predictions for this user tile ----
                # pred = ul . il with ul = ps2/z2 @ Wu^T + bu, reassociated as
                # (1/z2) * rowsum(ps2 * (il @ Wu)) + rowsum(il * bu) so the
                # tail is one fused DVE reduce straight off the pass-2 psum;
                # the H = il @ Wu chains hang off the early-ready item latents.
                il = lat_from(itemiN, tg, wit, None)
                jl = lat_from(itemjN, tg, wit, None)
                nc.gpsimd.tensor_add(il[:], il[:], bib[:])
                nc.gpsimd.tensor_add(jl[:], jl[:], bib[:])

                for lat2, pcols in ((il, pic), (jl, pjc)):
                    tp = tps.tile([128, 128], F32)
                    nc.tensor.transpose(tp[:], lat2[:], idt[:])
                    lT = latTp.tile([128, 128], F32)
                    nc.scalar.copy(lT[:], tp[:])
                    hp = mps.tile([128, D], F32, tag="ps", name="hp")
                    nc.tensor.matmul(hp[:], lhsT=lT[:], rhs=wu[:],
                                     start=True, stop=True)
                    hsb = prodp.tile([128, D], F32, tag="prodt", name="hsb")
                    nc.scalar.copy(hsb[:], hp[:])
                    scr = prodp.tile([128, D], F32, tag="prodt", name="scr")
                    praw = colp.tile([128, 1], F32)
                    nc.vector.affine_mul_reduce(
                        out=scr[:], accum_out=praw[:], in0=ps2[:], in1=hsb[:],
                        scale=1.0, bias=0.0,
                    )
                    pr = prodp.tile([128, FACTOR], F32, tag="predscratch")
                    pb = colp.tile([128, 1], F32)
                    nc.vector.affine_mul_reduce(
                        out=pr[:], accum_out=pb[:], in0=lat2[:], in1=bub[:],
                        scale=1.0, bias=0.0,
                    )
                    nc.vector.tensor_scalar_mul(pcols[:, tg:tg + 1], praw[:],
                                                r2c[:, tg:tg + 1])
                    nc.vector.tensor_add(pcols[:, tg:tg + 1],
                                         pcols[:, tg:tg + 1], pb[:])

            for tg in range(T_U):
                nc.sync.dma_start(itemi_o[128 * tg:128 * (tg + 1), :],
                                  itemiN[:, tg, :])
                nc.sync.dma_start(itemj_o[128 * tg:128 * (tg + 1), :],
                                  itemjN[:, tg, :])
            nc.sync.dma_start(pred_i_o[:], pic[:])
            nc.sync.dma_start(pred_j_o[:], pjc[:])

    nc.compile()
    return nc


def _pack_range(vrows, vsegs, seg_lo, seg_hi):
    """Collect the (already seg-sorted, in-range) piece rows covering segments
    [seg_lo, seg_hi), one list per 128-segment tile. Empty segments get one
    dummy zero-row (row index -1, so z=1 and the pooled vector is 0)."""
    ntiles = (seg_hi - seg_lo) // 128
    rows_t, srel_t = [], []
    for t in range(ntiles):
        s0 = seg_lo + 128 * t
        lo = np.searchsorted(vsegs, s0)
        hi = np.searchsorted(vsegs, s0 + 128)
        rows = vrows[lo:hi]
        srel = (vsegs[lo:hi] - s0).astype(np.float32)
        cnt = np.bincount(vsegs[lo:hi] - s0, minlength=128)
        emp = np.nonzero(cnt == 0)[0]
        rows = np.concatenate([rows, np.full(len(emp), -1, np.int64)])
        srel = np.concatenate([srel, emp.astype(np.float32)])
        rows_t.append(rows)
        srel_t.append(srel)
    return rows_t, srel_t


def _materialize(rows_t, srel_t, Xsrc, ch):
    """Lay packed tiles out as [ntiles*ch*128, D] row stream + [128, ntiles*ch]
    segrel columns, padding with segrel=-1 rows that contribute nothing."""
    ntiles = len(rows_t)
    idx = np.full((ntiles, ch * 128), -1, np.int64)
    srel = np.full((ntiles, ch * 128), -1.0, np.float32)
    for t in range(ntiles):
        n = len(rows_t[t])
        assert n <= ch * 128
        idx[t, :n] = rows_t[t]
        srel[t, :n] = srel_t[t]
    idx = idx.reshape(-1)
    xs = Xsrc[np.where(idx >= 0, idx, 0)]
    xs[idx < 0] = 0.0
    sr = np.ascontiguousarray(srel.reshape(-1, 128).T)   # [128, ntiles*ch]
    return np.ascontiguousarray(xs), sr


def _host_prep(inputs):
    upo = np.asarray(inputs["user_pooled_out"], dtype=np.float32)
    xj_full = np.asarray(inputs["itemj_piece_rep"], dtype=np.float32)
    Ws1 = np.asarray(inputs["Ws1"], dtype=np.float32)
    ws2 = np.asarray(inputs["ws2"], dtype=np.float32)
    Ws01 = np.asarray(inputs["Ws01"], dtype=np.float32)
    ws02 = np.asarray(inputs["ws02"], dtype=np.float32)
    user_W = np.asarray(inputs["user_W"], dtype=np.float32)
    user_b = np.asarray(inputs["user_b"], dtype=np.float32)
    item_W = np.asarray(inputs["item_W"], dtype=np.float32)
    item_b = np.asarray(inputs["item_b"], dtype=np.float32)
    piece_seg = np.asarray(inputs["piece_seg"]).astype(np.int64)
    user_seg = np.asarray(inputs["user_seg"]).astype(np.int64)
    itemj_seg = np.asarray(inputs["itemj_seg"]).astype(np.int64)
    pos = int(np.asarray(inputs["itemi_pos"]))

    # jax segment ops drop out-of-range ids (incl. the int32-overflow tail)
    pmask = (piece_seg >= 0) & (piece_seg < N_UNI)
    pvrows = np.nonzero(pmask)[0]
    pvsegs = piece_seg[pvrows]
    assert np.all(np.diff(pvsegs) >= 0), "valid piece_seg must be sorted"
    jmask = (itemj_seg >= 0) & (itemj_seg < BS)
    jvrows = np.nonzero(jmask)[0]
    jvsegs = itemj_seg[jvrows]
    assert np.all(np.diff(jvsegs) >= 0), "valid itemj_seg must be sorted"
    assert np.all((user_seg >= 0) & (user_seg < BS)) and \
        np.all(np.diff(user_seg) >= 0)
    assert np.bincount(user_seg, minlength=BS).min() >= 1
    assert 0 <= pos and pos + BS <= N_UNI

    v0 = (ws02.astype(np.float64)[0] @ Ws01.astype(np.float64)).astype(np.float32)
    v1 = (ws2.astype(np.float64)[0] @ Ws1.astype(np.float64)).astype(np.float32)

    WiT = np.stack([item_W[:, 128 * k:128 * (k + 1)].T for k in range(4)], axis=1)

    const = {
        "V0B": np.ascontiguousarray(np.broadcast_to(v0, (128, D))),
        "V1B": np.ascontiguousarray(np.broadcast_to(v1, (128, D))),
        "IOT": np.ascontiguousarray(
            np.broadcast_to(np.arange(128, dtype=np.float32), (128, 128))),
        "WU": np.ascontiguousarray(user_W),
        "WIT": np.ascontiguousarray(WiT),
        "BUB": np.ascontiguousarray(np.broadcast_to(user_b, (128, 128))),
        "BIB": np.ascontiguousarray(np.broadcast_to(item_b, (128, 128))),
        "IDT": np.eye(128, dtype=np.float32),
        "ONE": np.ones((128, 2), np.float32),
    }

    packs = []
    for c in range(NCORES):
        # this core's users -> uni rows must be exactly its uniform slice
        # (needed for SBUF locality of pass 2)
        u_lo = np.searchsorted(user_seg, U_C * c)
        u_hi = np.searchsorted(user_seg, U_C * (c + 1))
        assert u_lo == UNI_C * c and u_hi == UNI_C * (c + 1), \
            "user_seg must partition uni rows uniformly across cores"
        # per-tile alignment for pass 2 chunks
        sr2 = np.empty((UNI_C,), np.float32)
        for tg in range(T_U):
            blk = user_seg[UNI_C * c + UNI_PER_TILE * tg:
                           UNI_C * c + UNI_PER_TILE * (tg + 1)]
            rel = blk - (U_C * c + 128 * tg)
            assert rel.min() >= 0 and rel.max() < 128, \
                "user_seg tiles must align to 128-user blocks"
            sr2[UNI_PER_TILE * tg:UNI_PER_TILE * (tg + 1)] = rel
        sr2 = np.ascontiguousarray(sr2.reshape(T1, 128).T)

        p1 = _pack_range(pvrows, pvsegs, UNI_C * c, UNI_C * (c + 1))
        pi = _pack_range(pvrows, pvsegs, pos + U_C * c, pos + U_C * (c + 1))
        pj = _pack_range(jvrows, jvsegs, U_C * c, U_C * (c + 1))
        packs.append((p1, pi, pj, sr2))

    # SPMD: one program for all cores — pad every core to the max chunk count
    def maxch(i):
        return max(1, max(-(-len(r) // 128)
                          for p in packs for r in p[i][0]))

    ch1, chi, chj = maxch(0), maxch(1), maxch(2)

    in_maps = []
    for c in range(NCORES):
        p1, pi, pj, sr2 = packs[c]
        x1, sr1 = _materialize(p1[0], p1[1], upo, ch1)
        xi, sri = _materialize(pi[0], pi[1], upo, chi)
        xj, srj = _materialize(pj[0], pj[1], xj_full, chj)
        m = dict(const)
        m.update({"X1": x1, "SR1": sr1, "XI": xi, "SRI": sri,
                  "XJ": xj, "SRJ": srj, "SR2": sr2})
        in_maps.append(m)
    return in_maps, (ch1, chi, chj)


def kernel(**inputs):
    in_maps, chs = _host_prep(inputs)
    if chs not in _CACHE:
        _CACHE[chs] = _build_program(*chs)
    nc = _CACHE[chs]
    res = run_bass_kernel_spmd(nc, in_maps, core_ids=list(range(NCORES)))
    itemi = np.concatenate([res.results[c]["itemi_o"] for c in range(NCORES)], 0)
    itemj = np.concatenate([res.results[c]["itemj_o"] for c in range(NCORES)], 0)
    pred_i = np.concatenate([res.results[c]["pred_i_o"].T.reshape(-1) for c in range(NCORES)], 0)
    pred_j = np.concatenate([res.results[c]["pred_j_o"].T.reshape(-1) for c in range(NCORES)], 0)
    return itemi, itemj, pred_i, pred_j
